# revision 12
# baseline (speedup 1.0000x reference)
"""Cross-attention kernel for Trainium2 (8 NeuronCores, SPMD data-parallel).

Problem: O = softmax(Q @ K^T) @ V with B=4, Lq=Lk=4096, D=64, fp32 (no
1/sqrt(d) scaling).

Sharding: 8 cores = 4 batches x 2 Lq-halves. Each core handles a
[2048, 64] Q shard against the full [4096, 64] K/V of its batch.
Independent outputs -> no collectives.

Per-core algorithm (layouts chosen so nothing is transposed on-chip):
  - Host supplies QT [128, 2048] fp16 (D on partitions, duplicated into
    rows 64..127), KT [128, 4096] fp16 with rows 64..127 ZERO, and
    VA [4096, 128] bf16 = [V | ones | zeros].
  - The zero padding makes every matmul a full 128x128-activity op.
    TRN2's PE_HAM clock gate only un-throttles (1.2 -> 2.4 GHz) when the
    PE array's activity is high; half-array matmuls (contraction 64, or
    65 output partitions) never cross the threshold and the whole kernel
    runs at half clock. Padded operands cost the same cycles (cycles =
    moving-dim size) but register full activity -> warm clock.
  - ST[k, q] = matmul(lhsT=KTpad chunk [128,128], rhs=QT [128,512]);
    rows 64..127 of KTpad are zero so the duplicated QT rows contribute 0.
  - PT = exp(ST) on the scalar engine, written as bf16 (no max
    subtraction: |scores| < ~50, exp fits fp32/bf16 range; fp16 P would
    underflow). The scalar engine at 1 elem/cycle/lane is the kernel's
    bottleneck, so exp instructions are kept at 1024 free elements.
  - OT[128, q] += matmul(lhsT=VA chunk [128, 128] bf16, rhs=PT [128, 512]):
    rows 0..63 accumulate unnormalized output, row 64 the softmax
    denominator, rows 65..127 zeros. PT is consumed directly as rhs -
    no transpose anywhere.
  - Normalize: fast-reciprocal of row 64, gpsimd partition-broadcast,
    multiply, DMA out OT [64, 2048]; host transposes back.
"""

import sys

for _p in ("/opt/trn_rl_repo", "/opt/pypackages"):
    if _p not in sys.path:
        sys.path.insert(0, _p)

from contextlib import ExitStack

import ml_dtypes
import numpy as np

import concourse.bacc as bacc
import concourse.mybir as mybir
import concourse.tile as tile
from concourse.bass_utils import run_bass_kernel_spmd

# Problem constants (hardcoded per contract).
B, LQ, LK, D = 4, 4096, 4096, 64
N_CORES = 8
LQ_SHARD = LQ * B // N_CORES  # 2048
QB = 1024  # q-block (exp instruction free-size; 2 PSUM banks)
NQB = LQ_SHARD // QB  # 2
KC = 128  # k-chunk (contraction tile for the PV matmul)
NKC = LK // KC  # 32
SL = 512  # matmul moving-dim slice (one PSUM bank)
NSL = QB // SL  # 2

F32 = mybir.dt.float32
F16 = mybir.dt.float16
BF16 = mybir.dt.bfloat16

BF16NP = ml_dtypes.bfloat16

FAST_RECIP = True  # approx+NR reciprocal (~2 ULP) instead of exact (~6.5us)

KT_PIECE = 512  # kt DMA piece width (cols); 4 k-chunks per piece
VA_PIECE = 8  # va DMA piece size in k-chunks


def _build_program():
    nc = bacc.Bacc(
        "TRN2",
        target_bir_lowering=False,
        debug=False,
        num_devices=N_CORES,
    )
    qt_d = nc.declare_dram_parameter("QT", [2 * D, LQ_SHARD], F16, isOutput=False)
    kt_d = nc.declare_dram_parameter("KT", [2 * D, LK], F16, isOutput=False)
    # VA partition-major [p, c, d]: per-partition lines are contiguous 2KB+
    # (the natural [k, d] layout would scatter 256B lines and choke the DMA
    # queues for ~12us).
    va_d = nc.declare_dram_parameter("VA", [KC, NKC, KC], BF16, isOutput=False)
    ot_d = nc.declare_dram_parameter("OT", [D, LQ_SHARD], F32, isOutput=True)

    with tile.TileContext(nc) as tc, ExitStack() as ctx:
        singles = ctx.enter_context(tc.tile_pool(name="singles", bufs=1))
        st_pool = ctx.enter_context(tc.tile_pool(name="st", bufs=2, space="PSUM"))
        ot_pool = ctx.enter_context(tc.tile_pool(name="ot", bufs=2, space="PSUM"))
        pt_pool = ctx.enter_context(tc.tile_pool(name="pt", bufs=3))
        out_pool = ctx.enter_context(tc.tile_pool(name="out", bufs=2))
        norm_pool = ctx.enter_context(tc.tile_pool(name="norm", bufs=4))

        # Preload the exp activation table while input DMAs run.
        warm = singles.tile([1, 2], F32)
        nc.vector.memset(warm[:, :], 0.0)
        nc.scalar.activation(
            out=warm[:, :], in_=warm[:, :],
            func=mybir.ActivationFunctionType.Exp,
        )

        # Inputs are split into pieces so the first score matmuls don't
        # wait for the full 2.5 MB of loads.
        KH = LK // 2  # kt half width
        VH = NKC // 2  # va half size in chunks
        kt_sb = []
        qt_sb = []
        va_sb = []
        for h in range(2):
            tq = singles.tile([2 * D, QB], F16, name=f"qt{h}")
            for p in range(QB // SL):
                sl = slice(p * SL, (p + 1) * SL)
                sg = slice(h * QB + p * SL, h * QB + (p + 1) * SL)
                nc.sync.dma_start(out=tq[:, sl], in_=qt_d[:, sg])
            qt_sb.append(tq)
            t = singles.tile([2 * D, KH], F16, name=f"kt{h}")
            for p in range(KH // KT_PIECE):
                sl = slice(p * KT_PIECE, (p + 1) * KT_PIECE)
                sg = slice(h * KH + p * KT_PIECE, h * KH + (p + 1) * KT_PIECE)
                nc.sync.dma_start(out=t[:, sl], in_=kt_d[:, sg])
            kt_sb.append(t)
            tv = singles.tile([KC, VH, KC], BF16, name=f"va{h}")
            for p in range(VH // VA_PIECE):
                sl = slice(p * VA_PIECE, (p + 1) * VA_PIECE)
                sg = slice(h * VH + p * VA_PIECE, h * VH + (p + 1) * VA_PIECE)
                nc.sync.dma_start(out=tv[:, sl, :], in_=va_d[:, sg, :])
            va_sb.append(tv)

        # Warm the PE clock while input DMAs run: HAM un-throttles only
        # after ~3.4us of high-activity execution, so burn the DMA-wait
        # window on full-array dummy matmuls into a scratch PSUM tile.
        wz = singles.tile([KC, SL], BF16, name="wz")
        nc.vector.memset(wz[:, :], 0.0)
        warm_ps = st_pool.tile([KC, QB], F32, tag="st")
        for _ in range(14):
            nc.tensor.matmul(
                out=warm_ps[:, 0:SL],
                lhsT=wz[:, 0:KC],
                rhs=wz[:, :],
                start=True,
                stop=True,
            )

        def kt_ap(c):
            # [128, 128] fp16 weights for chunk c (rows 64..127 zero)
            t = kt_sb[c * KC // KH]
            off = (c * KC) % KH
            return t[:, off : off + KC]

        def va_ap(c):
            return va_sb[c // VH][:, c % VH, :]

        for qb in range(NQB):
            ot_ps = ot_pool.tile([KC, QB], F32)
            qt = qt_sb[qb]
            for c in range(NKC):
                st_ps = st_pool.tile([KC, QB], F32, tag="st")
                for s in range(NSL):
                    nc.tensor.matmul(
                        out=st_ps[:, s * SL : (s + 1) * SL],
                        lhsT=kt_ap(c),
                        rhs=qt[:, s * SL : (s + 1) * SL],
                        start=True,
                        stop=True,
                    )
                pt = pt_pool.tile([KC, QB], BF16)
                nc.scalar.activation(
                    out=pt[:, :],
                    in_=st_ps[:, :],
                    func=mybir.ActivationFunctionType.Exp,
                )
                for s in range(NSL):
                    nc.tensor.matmul(
                        out=ot_ps[:, s * SL : (s + 1) * SL],
                        lhsT=va_ap(c),
                        rhs=pt[:, s * SL : (s + 1) * SL],
                        start=(c == 0),
                        stop=(c == NKC - 1),
                    )
            # Normalize: O[d, q] = OT[d, q] / OT[64, q].  Per-512 slices so
            # the DVE recip, gpsimd broadcast, DVE multiply, and output DMA
            # pipeline across slices instead of serializing at the tail.
            for s in range(NSL):
                sl = slice(s * SL, (s + 1) * SL)
                den = norm_pool.tile([1, SL], F32)
                nc.vector.tensor_copy(den[:, :], ot_ps[D : D + 1, sl])
                recip = norm_pool.tile([1, SL], F32)
                scratch = norm_pool.tile([1, SL], F32)
                nc.vector.reciprocal_approx_accurate(
                    recip[:, :], den[:, :], scratch[:, :]
                )
                bcast = norm_pool.tile([D, SL], F32)
                nc.gpsimd.partition_broadcast(bcast[:, :], recip[:, :])
                o_sb = out_pool.tile([D, SL], F32)
                nc.vector.tensor_mul(o_sb[:, :], ot_ps[0:D, sl], bcast[:, :])
                nc.sync.dma_start(
                    out=ot_d[:, qb * QB + s * SL : qb * QB + (s + 1) * SL],
                    in_=o_sb[:, :],
                )

    nc.finalize()
    return nc


_PROGRAM_CACHE = {}


def _get_program():
    if "nc" not in _PROGRAM_CACHE:
        _PROGRAM_CACHE["nc"] = _build_program()
    return _PROGRAM_CACHE["nc"]


def _make_in_maps(Q, K, V):
    Q = np.asarray(Q, dtype=np.float32)
    K = np.asarray(K, dtype=np.float32)
    V = np.asarray(V, dtype=np.float32)
    in_maps = []
    for core in range(N_CORES):
        b, half = core // 2, core % 2
        q_shard = Q[b, half * LQ_SHARD : (half + 1) * LQ_SHARD, :]  # [2048, 64]
        qt1 = q_shard.T.astype(np.float16)  # [64, 2048]
        qt = np.concatenate([qt1, qt1], axis=0)  # [128, 2048] (dup rows)
        kt = np.zeros((2 * D, LK), dtype=np.float16)  # [128, 4096]
        kt[:D, :] = K[b].T.astype(np.float16)
        va = np.zeros((LK, KC), dtype=BF16NP)  # [4096, 128]
        va[:, :D] = V[b].astype(BF16NP)
        va[:, D] = 1.0
        # partition-major [p, c, d] so device DMA lines are contiguous
        # partition-major [p, c, d] so device DMA lines are contiguous
        va_pm = va.reshape(NKC, KC, KC).transpose(1, 0, 2)
        in_maps.append(
            {
                "QT": np.ascontiguousarray(qt),
                "KT": np.ascontiguousarray(kt),
                "VA": np.ascontiguousarray(va_pm),
            }
        )
    return in_maps


def _run(Q, K, V, trace=False, **spmd_kwargs):
    nc = _get_program()
    in_maps = _make_in_maps(Q, K, V)
    res = run_bass_kernel_spmd(
        nc, in_maps, list(range(N_CORES)), trace=trace, **spmd_kwargs
    )
    out = np.empty((B, LQ, D), dtype=np.float32)
    for core in range(N_CORES):
        b, half = core // 2, core % 2
        ot = res.results[core]["OT"]  # [64, 2048]
        out[b, half * LQ_SHARD : (half + 1) * LQ_SHARD, :] = ot.T
    return out, res


def kernel(Q, K, V):
    out, _ = _run(Q, K, V, trace=False)
    return out


# revision 13
# speedup vs baseline: 1.0127x; 1.0127x over previous
"""Cross-attention kernel for Trainium2 (8 NeuronCores, SPMD data-parallel).

Problem: O = softmax(Q @ K^T) @ V with B=4, Lq=Lk=4096, D=64, fp32 (no
1/sqrt(d) scaling).

Sharding: 8 cores = 4 batches x 2 Lq-halves. Each core handles a
[2048, 64] Q shard against the full [4096, 64] K/V of its batch.
Independent outputs -> no collectives.

Per-core algorithm (layouts chosen so nothing is transposed on-chip):
  - Host supplies QT [128, 2048] fp16 (D on partitions, duplicated into
    rows 64..127), KT [128, 4096] fp16 with rows 64..127 ZERO, and
    VA [4096, 128] bf16 = [V | ones | zeros].
  - The zero padding makes every matmul a full 128x128-activity op.
    TRN2's PE_HAM clock gate only un-throttles (1.2 -> 2.4 GHz) when the
    PE array's activity is high; half-array matmuls (contraction 64, or
    65 output partitions) never cross the threshold and the whole kernel
    runs at half clock. Padded operands cost the same cycles (cycles =
    moving-dim size) but register full activity -> warm clock.
  - ST[k, q] = matmul(lhsT=KTpad chunk [128,128], rhs=QT [128,512]);
    rows 64..127 of KTpad are zero so the duplicated QT rows contribute 0.
  - PT = exp(ST) on the scalar engine, written as bf16 (no max
    subtraction: |scores| < ~50, exp fits fp32/bf16 range; fp16 P would
    underflow). The scalar engine at 1 elem/cycle/lane is the kernel's
    bottleneck, so exp instructions are kept at 1024 free elements.
  - OT[128, q] += matmul(lhsT=VA chunk [128, 128] bf16, rhs=PT [128, 512]):
    rows 0..63 accumulate unnormalized output, row 64 the softmax
    denominator, rows 65..127 zeros. PT is consumed directly as rhs -
    no transpose anywhere.
  - Normalize: fast-reciprocal of row 64, gpsimd partition-broadcast,
    multiply, DMA out OT [64, 2048]; host transposes back.
"""

import sys

for _p in ("/opt/trn_rl_repo", "/opt/pypackages"):
    if _p not in sys.path:
        sys.path.insert(0, _p)

from contextlib import ExitStack

import ml_dtypes
import numpy as np

import concourse.bacc as bacc
import concourse.mybir as mybir
import concourse.tile as tile
from concourse.bass_utils import run_bass_kernel_spmd

# Problem constants (hardcoded per contract).
B, LQ, LK, D = 4, 4096, 4096, 64
N_CORES = 8
LQ_SHARD = LQ * B // N_CORES  # 2048
QB = 1024  # q-block (exp instruction free-size; 2 PSUM banks)
NQB = LQ_SHARD // QB  # 2
KC = 128  # k-chunk (contraction tile for the PV matmul)
NKC = LK // KC  # 32
SL = 512  # matmul moving-dim slice (one PSUM bank)
NSL = QB // SL  # 2

F32 = mybir.dt.float32
F16 = mybir.dt.float16
BF16 = mybir.dt.bfloat16

BF16NP = ml_dtypes.bfloat16

FAST_RECIP = True  # approx+NR reciprocal (~2 ULP) instead of exact (~6.5us)

KT_PIECE = 512  # kt DMA piece width (cols); 4 k-chunks per piece
VA_PIECE = 8  # va DMA piece size in k-chunks


def _build_program():
    nc = bacc.Bacc(
        "TRN2",
        target_bir_lowering=False,
        debug=False,
        num_devices=N_CORES,
    )
    qt_d = nc.declare_dram_parameter("QT", [2 * D, LQ_SHARD], F16, isOutput=False)
    kt_d = nc.declare_dram_parameter("KT", [2 * D, LK], F16, isOutput=False)
    # VA partition-major [p, c, d]: per-partition lines are contiguous 2KB+
    # (the natural [k, d] layout would scatter 256B lines and choke the DMA
    # queues for ~12us).
    va_d = nc.declare_dram_parameter("VA", [KC, NKC, KC], BF16, isOutput=False)
    ot_d = nc.declare_dram_parameter("OT", [D, LQ_SHARD], F32, isOutput=True)

    with tile.TileContext(nc) as tc, ExitStack() as ctx:
        singles = ctx.enter_context(tc.tile_pool(name="singles", bufs=1))
        st_pool = ctx.enter_context(tc.tile_pool(name="st", bufs=2, space="PSUM"))
        ot_pool = ctx.enter_context(tc.tile_pool(name="ot", bufs=2, space="PSUM"))
        pt_pool = ctx.enter_context(tc.tile_pool(name="pt", bufs=3))
        out_pool = ctx.enter_context(tc.tile_pool(name="out", bufs=2))
        norm_pool = ctx.enter_context(tc.tile_pool(name="norm", bufs=4))

        # Preload the exp activation table while input DMAs run.
        warm = singles.tile([1, 2], F32)
        nc.vector.memset(warm[:, :], 0.0)
        nc.scalar.activation(
            out=warm[:, :], in_=warm[:, :],
            func=mybir.ActivationFunctionType.Exp,
        )

        # Inputs are split into pieces so the first score matmuls don't
        # wait for the full 2.5 MB of loads.
        KH = LK // 2  # kt half width
        VH = NKC // 2  # va half size in chunks
        kt_sb = []
        qt_sb = []
        va_sb = []
        for h in range(2):
            tq = singles.tile([2 * D, QB], F16, name=f"qt{h}")
            for p in range(QB // SL):
                sl = slice(p * SL, (p + 1) * SL)
                sg = slice(h * QB + p * SL, h * QB + (p + 1) * SL)
                nc.sync.dma_start(out=tq[:, sl], in_=qt_d[:, sg])
            qt_sb.append(tq)
            t = singles.tile([2 * D, KH], F16, name=f"kt{h}")
            for p in range(KH // KT_PIECE):
                sl = slice(p * KT_PIECE, (p + 1) * KT_PIECE)
                sg = slice(h * KH + p * KT_PIECE, h * KH + (p + 1) * KT_PIECE)
                nc.sync.dma_start(out=t[:, sl], in_=kt_d[:, sg])
            kt_sb.append(t)
            tv = singles.tile([KC, VH, KC], BF16, name=f"va{h}")
            for p in range(VH // VA_PIECE):
                sl = slice(p * VA_PIECE, (p + 1) * VA_PIECE)
                sg = slice(h * VH + p * VA_PIECE, h * VH + (p + 1) * VA_PIECE)
                nc.sync.dma_start(out=tv[:, sl, :], in_=va_d[:, sg, :])
            va_sb.append(tv)

        def kt_ap(c):
            # [128, 128] fp16 weights for chunk c (rows 64..127 zero)
            t = kt_sb[c * KC // KH]
            off = (c * KC) % KH
            return t[:, off : off + KC]

        def va_ap(c):
            return va_sb[c // VH][:, c % VH, :]

        for qb in range(NQB):
            ot_ps = ot_pool.tile([KC, QB], F32)
            qt = qt_sb[qb]
            for c in range(NKC):
                st_ps = st_pool.tile([KC, QB], F32, tag="st")
                for s in range(NSL):
                    nc.tensor.matmul(
                        out=st_ps[:, s * SL : (s + 1) * SL],
                        lhsT=kt_ap(c),
                        rhs=qt[:, s * SL : (s + 1) * SL],
                        start=True,
                        stop=True,
                    )
                pt = pt_pool.tile([KC, QB], BF16)
                nc.scalar.activation(
                    out=pt[:, :],
                    in_=st_ps[:, :],
                    func=mybir.ActivationFunctionType.Exp,
                )
                for s in range(NSL):
                    nc.tensor.matmul(
                        out=ot_ps[:, s * SL : (s + 1) * SL],
                        lhsT=va_ap(c),
                        rhs=pt[:, s * SL : (s + 1) * SL],
                        start=(c == 0),
                        stop=(c == NKC - 1),
                    )
            # Normalize: O[d, q] = OT[d, q] / OT[64, q].  Per-512 slices,
            # with all DVE recip work issued before the multiplies so the
            # gpsimd broadcasts overlap the other slice's recip chain
            # (DVE executes its queue in order).
            recips = []
            for s in range(NSL):
                sl = slice(s * SL, (s + 1) * SL)
                den = norm_pool.tile([1, SL], F32)
                nc.vector.tensor_copy(den[:, :], ot_ps[D : D + 1, sl])
                recip = norm_pool.tile([1, SL], F32)
                scratch = norm_pool.tile([1, SL], F32)
                nc.vector.reciprocal_approx_accurate(
                    recip[:, :], den[:, :], scratch[:, :]
                )
                recips.append(recip)
            bcasts = []
            for s in range(NSL):
                bcast = norm_pool.tile([D, SL], F32)
                nc.gpsimd.partition_broadcast(bcast[:, :], recips[s][:, :])
                bcasts.append(bcast)
            for s in range(NSL):
                sl = slice(s * SL, (s + 1) * SL)
                o_sb = out_pool.tile([D, SL], F32)
                nc.vector.tensor_mul(o_sb[:, :], ot_ps[0:D, sl], bcasts[s][:, :])
                nc.sync.dma_start(
                    out=ot_d[:, qb * QB + s * SL : qb * QB + (s + 1) * SL],
                    in_=o_sb[:, :],
                )

    nc.finalize()
    return nc


_PROGRAM_CACHE = {}


def _get_program():
    if "nc" not in _PROGRAM_CACHE:
        _PROGRAM_CACHE["nc"] = _build_program()
    return _PROGRAM_CACHE["nc"]


def _make_in_maps(Q, K, V):
    Q = np.asarray(Q, dtype=np.float32)
    K = np.asarray(K, dtype=np.float32)
    V = np.asarray(V, dtype=np.float32)
    in_maps = []
    for core in range(N_CORES):
        b, half = core // 2, core % 2
        q_shard = Q[b, half * LQ_SHARD : (half + 1) * LQ_SHARD, :]  # [2048, 64]
        qt1 = q_shard.T.astype(np.float16)  # [64, 2048]
        qt = np.concatenate([qt1, qt1], axis=0)  # [128, 2048] (dup rows)
        kt = np.zeros((2 * D, LK), dtype=np.float16)  # [128, 4096]
        kt[:D, :] = K[b].T.astype(np.float16)
        va = np.zeros((LK, KC), dtype=BF16NP)  # [4096, 128]
        va[:, :D] = V[b].astype(BF16NP)
        va[:, D] = 1.0
        # partition-major [p, c, d] so device DMA lines are contiguous
        # partition-major [p, c, d] so device DMA lines are contiguous
        va_pm = va.reshape(NKC, KC, KC).transpose(1, 0, 2)
        in_maps.append(
            {
                "QT": np.ascontiguousarray(qt),
                "KT": np.ascontiguousarray(kt),
                "VA": np.ascontiguousarray(va_pm),
            }
        )
    return in_maps


def _run(Q, K, V, trace=False, **spmd_kwargs):
    nc = _get_program()
    in_maps = _make_in_maps(Q, K, V)
    res = run_bass_kernel_spmd(
        nc, in_maps, list(range(N_CORES)), trace=trace, **spmd_kwargs
    )
    out = np.empty((B, LQ, D), dtype=np.float32)
    for core in range(N_CORES):
        b, half = core // 2, core % 2
        ot = res.results[core]["OT"]  # [64, 2048]
        out[b, half * LQ_SHARD : (half + 1) * LQ_SHARD, :] = ot.T
    return out, res


def kernel(Q, K, V):
    out, _ = _run(Q, K, V, trace=False)
    return out
